# revision 15
# baseline (speedup 1.0000x reference)
"""Trainium2 Bass kernel for nn_DualPGD_3092376453437.

Math: the reference's 30-iteration PGD loop collapses in exact arithmetic.
The normalized Sylvester Hadamard Hmat is symmetric and involutive
(H = H^T, H @ H = I), so measure_H == adjoint_H == F with F(Z) = H Z H and
F(F(Z)) = Z.  With GAMMA = 1 the data-fidelity gradient step

    xk <- xk - F(F(xk) - m) = x0          (exact, every iteration)

resets xk to the pseudo-inverse init x0 = F(m), so the TV dual state u sees
the same gradient every iteration and the loop has a closed form.  Further,

    m  = 2*F(w) - F(ones),  w = (x+1)/2
    x0 = F(m) = 2*F(F(w)) - F(F(ones)) = 2*w - ones     (exact)
    z  = (x0 + 1)/2 = w

so z equals w EXACTLY in real arithmetic -- every Hadamard multiply cancels.
The reference's z differs from w only by its own fp32-matmul rounding noise;
computing z := w on device is therefore CLOSER to the fp32 reference than
re-doing the transforms in fp32 (measured: 7.9e-5 absmax on output scale
9.3, i.e. ~8.5e-6 relative -- the fp32 noise floor).  Final closed form
(TAU = 0.25, 30*TAU = 7.5; folded by 2x so w is never materialized):

    vx  = clip(7.5 * D @ x, -2, 2)          (= 2*u_x;  D = row fwd-diff)
    vy  = clip(7.5 * gy(x), -2, 2)          (= 2*u_y;  free-dim fwd-diff)
    out = x - D^T @ vx - (vy - shift_right(vy))

On-device mapping (phased emission: DMA-in, dense vx-matmul burst,
elementwise, dense ax-matmul burst, combine -- dense PE bursts avoid both
in-order PE-queue stalls on the clip and HAM clock-gate re-throttling):
  - row-direction stencils are PE matmuls with the constant bidiagonal D:
    vx via lhsT = 7.5*D^T (out = lhsT^T @ x = 7.5*D @ x), the adjoint via
    lhsT = D.  The all-zero 128x128 block of D is skipped (3 matmuls per
    pass, each [K=128, M=128, N=256], fp32).
  - column-direction stencils are free-dim-offset vector ops (x75 = 7.5*x
    on ScalarE, shifted diff + clip + combine on VectorE/GpSimd).
  - cost-model timeline: ~45 us per core (PE 32 us busy, DVE 20, GpSimd 17).

Sharding: pure data parallel, 8 images per core on 8 NeuronCores.
"""

import numpy as np

import concourse.mybir as mybir
from concourse import bacc
from concourse.bass_utils import run_bass_kernel_spmd
from concourse.tile import TileContext

N_CORES = 8
IMGS = 8  # images per core
P = 128
W = 256
F32 = mybir.dt.float32

_CACHE: dict = {}


def _build():
    nc = bacc.Bacc("TRN2", target_bir_lowering=False, debug=False)

    x_d = nc.dram_tensor("x", [IMGS, W, W], F32, kind="ExternalInput").ap()
    # Hmat is unused by the collapsed algorithm but kept as an input so the
    # binding matches setup_inputs().
    H_d = nc.dram_tensor("Hmat", [W, W], F32, kind="ExternalInput").ap()
    DT75_d = nc.dram_tensor("DT75", [W, W], F32, kind="ExternalInput").ap()
    D_d = nc.dram_tensor("Dmat", [W, W], F32, kind="ExternalInput").ap()
    out_d = nc.dram_tensor("out", [IMGS, W, W], F32, kind="ExternalOutput").ap()

    # row r = po*128 + pi  ->  SBUF layout [pi, po, (img,) w]
    rc = lambda ap: ap.rearrange("(po pi) w -> pi po w", pi=P)
    Copy = mybir.ActivationFunctionType.Copy
    Alu = mybir.AluOpType

    with TileContext(nc) as tc:
        with (
            tc.tile_pool(name="const", bufs=1) as cpool,
            tc.tile_pool(name="sbuf", bufs=1) as pool,
            tc.tile_pool(name="psum", bufs=8, space="PSUM") as ppool,
        ):
            DT75_sb = cpool.tile([P, 2, W], F32, tag="DT75")
            D_sb = cpool.tile([P, 2, W], F32, tag="D")
            Hu_sb = cpool.tile([P, 2, W], F32, tag="Hu")  # unused load
            nc.sync.dma_start(DT75_sb, rc(DT75_d))
            nc.sync.dma_start(D_sb, rc(D_d))
            nc.sync.dma_start(Hu_sb, rc(H_d))

            def G_stencil(lhs_sb, dst_ps, rhs_sb, skip):
                # dst = lhs^T @ rhs; skip the all-zero (m,k) block
                for m in range(2):
                    ks = [k for k in range(2) if (m, k) != skip]
                    for j, k in enumerate(ks):
                        nc.tensor.matmul(
                            dst_ps[:, m, :],
                            lhs_sb[:, k, m * P:(m + 1) * P],
                            rhs_sb[:, k, :],
                            start=(j == 0),
                            stop=(j == len(ks) - 1),
                        )

            x_sbs, x75s, vxps, vxs, vys, axps = [], [], [], [], [], []

            # phase 1: all input DMAs
            for p in range(IMGS):
                x_sb = pool.tile([P, 2, W], F32, tag=f"x{p}")
                nc.sync.dma_start(x_sb, rc(x_d[p]))
                x_sbs.append(x_sb)

            # phase 2: dense vx matmul burst (keeps PE warm, no PE stalls)
            for p in range(IMGS):
                vxp = ppool.tile([P, 2, W], F32, tag="u")
                G_stencil(DT75_sb, vxp, x_sbs[p], skip=(1, 0))
                vxps.append(vxp)

            # phase 3: elementwise (x75, clips, vy) overlapping the bursts
            for p in range(IMGS):
                x75 = pool.tile([P, 2, W], F32, tag=f"x75_{p}")
                nc.scalar.activation(x75, x_sbs[p], Copy, bias=0.0, scale=7.5)
                x75s.append(x75)

                vx = pool.tile([P, 2, W], F32, tag=f"vx{p}")
                nc.vector.tensor_scalar(vx, vxps[p], -2.0, 2.0,
                                        op0=Alu.max, op1=Alu.min)
                vxs.append(vx)

                vy = pool.tile([P, 2, W], F32, tag=f"vy{p}")
                nc.vector.tensor_sub(
                    vy[:, :, 0:W - 1], x75[:, :, 1:W], x75[:, :, 0:W - 1]
                )
                nc.gpsimd.memset(vy[:, :, W - 1:W], 0.0)
                nc.gpsimd.tensor_scalar(vy, vy, -2.0, 2.0,
                                        op0=Alu.max, op1=Alu.min)
                vys.append(vy)

            # phase 4: dense ax matmul burst
            for p in range(IMGS):
                axp = ppool.tile([P, 2, W], F32, tag="u")
                G_stencil(D_sb, axp, vxs[p], skip=(0, 1))
                axps.append(axp)

            # phase 5: combine + output DMAs
            for p in range(IMGS):
                A = pool.tile([P, 2, W], F32, tag=f"A{p}")
                nc.vector.scalar_tensor_tensor(A, axps[p], -1.0, x_sbs[p],
                                               op0=Alu.mult, op1=Alu.add)
                nc.gpsimd.tensor_add(A, A, vys[p])
                nc.vector.tensor_sub(
                    A[:, :, 1:W], A[:, :, 1:W], vys[p][:, :, 0:W - 1]
                )
                nc.sync.dma_start(rc(out_d[p]), A)

    nc.compile()
    return nc


def _consts():
    D = np.zeros((W, W), np.float32)
    for i in range(W - 1):
        D[i, i] = -1.0
        D[i, i + 1] = 1.0
    DT75 = np.ascontiguousarray((7.5 * D.T).astype(np.float32))
    return D, DT75


def _in_maps(x, Hmat):
    xf = np.ascontiguousarray(np.asarray(x, np.float32).reshape(-1, W, W))
    Hm = np.ascontiguousarray(np.asarray(Hmat, np.float32))
    D, DT75 = _consts()
    per = xf.shape[0] // N_CORES
    return [
        {"x": xf[i * per:(i + 1) * per], "Hmat": Hm, "DT75": DT75, "Dmat": D}
        for i in range(N_CORES)
    ]


def kernel(x: np.ndarray, Hmat: np.ndarray) -> np.ndarray:
    if "nc" not in _CACHE:
        _CACHE["nc"] = _build()
    res = run_bass_kernel_spmd(_CACHE["nc"], _in_maps(x, Hmat), list(range(N_CORES)))
    out = np.concatenate([res.results[i]["out"] for i in range(N_CORES)], axis=0)
    return np.ascontiguousarray(out.reshape(x.shape).astype(np.float32))


def profile(np_inputs, tmpdir=None):
    """Run once with NTFF tracing; returns exec_time_ns (or None)."""
    if "nc" not in _CACHE:
        _CACHE["nc"] = _build()
    res = run_bass_kernel_spmd(
        _CACHE["nc"], _in_maps(np_inputs["x"], np_inputs["Hmat"]),
        list(range(N_CORES)), trace=True, tmpdir=tmpdir,
    )
    return res.exec_time_ns
